# revision 6
# baseline (speedup 1.0000x reference)
"""Depthwise causal Conv1d (B=4, S=4096, D=2048, K=4) on 8 TRN2 NeuronCores.

Sharding: 8 cores = batch(4) x sequence-halves(2); zero communication.
Each core gets a channel-major bf16 slab x_core[D, 3 + S/2] (3 history
columns: zeros at sequence start, else the previous half's tail) and
computes out[d, s] = sum_k w[d, k] * x[d, s - 3 + k] + bias[d].

The 4-tap accumulation runs on the TensorEngine (not hit by the TRN2
SBUF-src 2.3x errata) as diagonal matmuls accumulating in PSUM: stationary
diag(w[block, k]) [128x128], moving = a column window of the x tile. bf16
moving operands stream 2 cols/cycle but require 4-byte-aligned (even
element) start offsets, which odd taps k=1,3 violate; since PSUM is fp32
(4-byte elements), the odd shift is absorbed by the PSUM destination AP
instead: odd taps use moving start C+k+1 (even) and write pt[:, 1:N],
even taps write pt[:, 0:N], so pt[m] consistently accumulates out[C+m].

Per 128-channel block (free dim = 2048 outputs):
  - chunks C in {0, 510, 1020, 1530} (psum bank = 512 f32) + tail (2040, 8);
    4 matmuls each; combine pt[1:N]+bias -> out[C+1:C+N] split across
    ACT activation (3 big chunks) / DVE tensor_scalar (1 big + tail),
    both PSUM-src with bf16 out
  - out column 0 (needs pt[-1], unreachable) via a 1-col DVE chain
Inputs ride the SP HWDGE ring, outputs + weights the ACT ring.

bf16 I/O halves HBM traffic (the roofline for this memory-bound problem);
products accumulate in fp32 PSUM. Measured rel err ~5e-3 vs the fp32
reference (gate 2e-2).
"""

import numpy as np

import concourse.bacc as bacc
import concourse.mybir as mybir
from concourse.bass_utils import run_bass_kernel_spmd
from concourse.tile import TileContext

B, S, D, K = 4, 4096, 2048, 4
NCORES = 8
SHALF = S // 2          # 2048 sequence positions per core
HIST = K - 1            # 3 history columns
NBLK = D // 128         # 16 channel blocks
F32 = mybir.dt.float32
BF16 = mybir.dt.bfloat16
MULT = mybir.AluOpType.mult
ADD = mybir.AluOpType.add
# psum-bank chunks: (start col, even-tap width); odd taps/combine use NE-1
# for the tail so out[2047] is covered (one small odd-width matmul), NE-?
# big chunks use odd width 511 = NE-1 as well (measured no worse than 510)
CHUNKS = [(0, 512), (510, 512), (1020, 512), (1530, 512), (2040, 8)]

_CACHE = {}


def _emit_pass(nc, tc, pools, aps):
    x_d, o_d, wsb, wdg = aps
    xpool, ppool, opool, tpool = pools

    def diag(k, blk):
        c = k * NBLK + blk
        return wdg[:, c * 128 : (c + 1) * 128]

    def wcol(k, blk):
        return wsb[:, k * NBLK + blk : k * NBLK + blk + 1]

    def bias(blk):
        return wsb[:, K * NBLK + blk : K * NBLK + blk + 1]

    for blk in range(NBLK):
        xt = xpool.tile([128, SHALF + HIST], BF16, tag="xt")
        nc.sync.dma_start(out=xt[:], in_=x_d[blk * 128 : (blk + 1) * 128, :])
        ot = opool.tile([128, SHALF], BF16, tag="ot")

        # out[:, 0] = sum_k w_k * xt[:, k] + bias (f32 temp chain on DVE)
        tmp = tpool.tile([128, 1], F32, tag="tmp")
        nc.vector.scalar_tensor_tensor(
            tmp[:], xt[:, 3:4], wcol(3, blk), bias(blk), MULT, ADD
        )
        nc.vector.scalar_tensor_tensor(
            tmp[:], xt[:, 2:3], wcol(2, blk), tmp[:], MULT, ADD
        )
        nc.vector.scalar_tensor_tensor(
            tmp[:], xt[:, 1:2], wcol(1, blk), tmp[:], MULT, ADD
        )
        nc.vector.scalar_tensor_tensor(
            ot[:, 0:1], xt[:, 0:1], wcol(0, blk), tmp[:], MULT, ADD
        )

        for ci, (C, NE) in enumerate(CHUNKS):
            NO = NE - 1
            pt = ppool.tile([128, 512], F32, tag="pt")
            for j, k in enumerate((0, 2, 1, 3)):
                if k % 2 == 0:
                    nc.tensor.matmul(
                        pt[:, 0:NE],
                        diag(k, blk),
                        xt[:, C + k : C + k + NE],
                        start=(j == 0),
                        stop=(j == K - 1),
                    )
                else:
                    nc.tensor.matmul(
                        pt[:, 1 : 1 + NO],
                        diag(k, blk),
                        xt[:, C + k + 1 : C + k + 1 + NO],
                        start=False,
                        stop=(j == K - 1),
                    )
            osl = ot[:, C + 1 : C + 1 + NO]
            psl = pt[:, 1 : 1 + NO]
            # ACT takes 3 big chunks, DVE one big + the tail: DVE also
            # carries the col-0 chains, and ACT's PSUM-src activation is
            # cheaper per element than DVE's PSUM-src tensor_scalar
            if ci in (2, 4):
                nc.vector.tensor_scalar_add(osl, psl, bias(blk))
            else:
                nc.scalar.add(osl, psl, bias(blk))
        nc.scalar.dma_start(out=o_d[blk * 128 : (blk + 1) * 128, :], in_=ot[:])


def _build_program(nreps=1):
    """nreps passes of the kernel body, fully unrolled (nreps > 1 is used
    only by test.py for steady-state timing)."""
    key = nreps
    if key in _CACHE:
        return _CACHE[key]
    nc = bacc.Bacc("TRN2", num_devices=NCORES)
    x_d = nc.dram_tensor("xin", [D, SHALF + HIST], BF16, kind="ExternalInput").ap()
    w_d = nc.dram_tensor("wtab", [128, (K + 1) * NBLK], F32, kind="ExternalInput").ap()
    wd_d = nc.dram_tensor(
        "wdiag", [128, K * NBLK * 128], BF16, kind="ExternalInput"
    ).ap()
    o_d = nc.dram_tensor("out", [D, SHALF], BF16, kind="ExternalOutput").ap()

    with TileContext(nc) as tc:
        with (
            tc.tile_pool(name="const", bufs=1) as const,
            tc.tile_pool(name="xpool", bufs=6) as xpool,
            tc.psum_pool(name="ppool", bufs=8) as ppool,
            tc.tile_pool(name="opool", bufs=5) as opool,
            tc.tile_pool(name="tpool", bufs=4) as tpool,
        ):
            wsb = const.tile([128, (K + 1) * NBLK], F32, tag="wsb")
            nc.scalar.dma_start(out=wsb[:], in_=w_d)
            wdg = const.tile([128, K * NBLK * 128], BF16, tag="wdg")
            nc.scalar.dma_start(out=wdg[:], in_=wd_d)

            pools = (xpool, ppool, opool, tpool)
            aps = (x_d, o_d, wsb, wdg)
            for _ in range(nreps):
                _emit_pass(nc, tc, pools, aps)

    nc.compile()
    _CACHE[key] = nc
    return nc


def _shard_inputs(x, weight, bias):
    import ml_dtypes

    bf16 = ml_dtypes.bfloat16
    x = np.asarray(x, dtype=np.float32)
    weight = np.asarray(weight, dtype=np.float32)
    bias = np.asarray(bias, dtype=np.float32)

    wr = weight[:, 0, :].reshape(NBLK, 128, K)          # [blk, p, k]
    # wtab[p, k*NBLK+blk] = w[blk*128+p, k]; wtab[p, K*NBLK+blk] = bias
    wtab = np.empty((128, (K + 1) * NBLK), dtype=np.float32)
    wtab[:, : K * NBLK] = wr.transpose(1, 2, 0).reshape(128, K * NBLK)
    wtab[:, K * NBLK :] = bias.reshape(NBLK, 128).T
    # wdiag[p, (k*NBLK+blk)*128 + j] = w[blk*128+p, k] * (j == p)
    wd = np.zeros((128, K * NBLK, 128), dtype=np.float32)
    pidx = np.arange(128)
    for k in range(K):
        for blk in range(NBLK):
            wd[pidx, k * NBLK + blk, pidx] = wr[blk, :, k]
    wdiag = wd.reshape(128, K * NBLK * 128).astype(bf16)

    in_maps = []
    for core in range(NCORES):
        b, h = divmod(core, 2)
        s0 = h * SHALF
        xc = np.empty((D, SHALF + HIST), dtype=bf16)
        xbt = x[b].T  # [D, S] view
        if s0 == 0:
            xc[:, :HIST] = 0.0
            xc[:, HIST:] = xbt[:, :SHALF]
        else:
            xc[:] = xbt[:, s0 - HIST : s0 + SHALF]
        in_maps.append({"xin": xc, "wtab": wtab, "wdiag": wdiag})
    return in_maps


def _run(x, weight, bias, trace=False):
    nc = _build_program()
    in_maps = _shard_inputs(x, weight, bias)
    res = run_bass_kernel_spmd(nc, in_maps, list(range(NCORES)), trace=trace)
    out = np.empty((B, S, D), dtype=np.float32)
    for core in range(NCORES):
        b, h = divmod(core, 2)
        out[b, h * SHALF : (h + 1) * SHALF, :] = (
            res.results[core]["out"].astype(np.float32).T
        )
    return out, res


def kernel(x, weight, bias):
    out, _ = _run(x, weight, bias, trace=False)
    return out


# revision 7
# speedup vs baseline: 1.4115x; 1.4115x over previous
"""Depthwise causal Conv1d (B=4, S=4096, D=2048, K=4) on 8 TRN2 NeuronCores.

Sharding: 8 cores = batch(4) x sequence-halves(2); zero communication.
Each core gets a channel-major bf16 slab x_core[D, 4 + S/2] (4 history
columns: zeros at sequence start, else the previous half's tail) and
computes out[d, s] = sum_k w[d, k] * x[d, s - 3 + k] + bias[d], i.e. with
xt[t] = x[t - 4]:  out[p] = sum_k w_k * xt[p + k + 1].

The 4-tap accumulation runs on the TensorEngine (not hit by the TRN2
SBUF-src 2.3x errata) as diagonal matmuls accumulating in PSUM: stationary
diag(w[block, k]) [128x128] bf16, moving = a column window of the x tile.
bf16 moving operands stream 2 cols/cycle but require 4-byte-aligned (even
element) start offsets; since PSUM is fp32 (4-byte elements), odd shifts
are absorbed by the PSUM destination AP instead of the moving AP. With
ODD chunk bases C, even taps use moving start C+k+1 (even) writing
pt[:, 0:512] and odd taps use moving start C+k+2 (even) writing
pt[:, 1:512], so pt[m] consistently accumulates out[C+m]:

  chunks C in {-1, 509, 1019, 1529} (one 512-f32 PSUM bank each) plus a
  9-col tail at C=2039; combine pt[1:1+N]+bias -> out[C+1 .. C+N] split
  across ACT activation (3 big chunks) and DVE tensor_scalar (1 big +
  tail), both PSUM-src with bf16 out. Chunk C=-1 reaches out[0], so no
  scalar edge fix-ups are needed anywhere.

Inputs ride the SP HWDGE ring, outputs + weights the ACT ring. bf16 I/O
halves HBM traffic (the roofline for this memory-bound problem); products
accumulate in fp32 PSUM. Measured rel err ~5e-3 vs the fp32 reference
(gate 2e-2).
"""

import numpy as np

import concourse.bacc as bacc
import concourse.mybir as mybir
from concourse.bass_utils import run_bass_kernel_spmd
from concourse.tile import TileContext

B, S, D, K = 4, 4096, 2048, 4
NCORES = 8
SHALF = S // 2          # 2048 sequence positions per core
HIST = 4                # history columns (K-1 needed + 1 alignment pad)
NBLK = D // 128         # 16 channel blocks
F32 = mybir.dt.float32
BF16 = mybir.dt.bfloat16
# (odd chunk base C, even-tap matmul width); odd taps/combine use width-1
CHUNKS = [(-1, 512), (509, 512), (1019, 512), (1529, 512), (2039, 9)]

_CACHE = {}


def _emit_pass(nc, pools, aps):
    x_d, o_d, wsb, wdg = aps
    xpool, ppool, opool = pools

    def diag(k, blk):
        c = k * NBLK + blk
        return wdg[:, c * 128 : (c + 1) * 128]

    def bias(blk):
        return wsb[:, K * NBLK + blk : K * NBLK + blk + 1]

    for blk in range(NBLK):
        xt = xpool.tile([128, SHALF + HIST], BF16, tag="xt")
        nc.sync.dma_start(out=xt[:], in_=x_d[blk * 128 : (blk + 1) * 128, :])
        ot = opool.tile([128, SHALF], BF16, tag="ot")
        for ci, (C, NE) in enumerate(CHUNKS):
            NO = NE - 1
            pt = ppool.tile([128, 512], F32, tag="pt")
            for j, k in enumerate((0, 2, 1, 3)):
                if k % 2 == 0:
                    nc.tensor.matmul(
                        pt[:, 0:NE],
                        diag(k, blk),
                        xt[:, C + k + 1 : C + k + 1 + NE],
                        start=(j == 0),
                        stop=(j == K - 1),
                    )
                else:
                    nc.tensor.matmul(
                        pt[:, 1 : 1 + NO],
                        diag(k, blk),
                        xt[:, C + k + 2 : C + k + 2 + NO],
                        start=False,
                        stop=(j == K - 1),
                    )
            osl = ot[:, C + 1 : C + 1 + NO]
            psl = pt[:, 1 : 1 + NO]
            # ACT takes 3 big chunks, DVE one big + the tail: ACT's
            # PSUM-src activation is cheaper per element than DVE's
            # PSUM-src tensor_scalar
            if ci in (2, 4):
                nc.vector.tensor_scalar_add(osl, psl, bias(blk))
            else:
                nc.scalar.add(osl, psl, bias(blk))
        nc.scalar.dma_start(out=o_d[blk * 128 : (blk + 1) * 128, :], in_=ot[:])


def _build_program(nreps=1):
    """nreps passes of the kernel body, fully unrolled (nreps > 1 is used
    only by test.py for steady-state timing)."""
    if nreps in _CACHE:
        return _CACHE[nreps]
    nc = bacc.Bacc("TRN2", num_devices=NCORES)
    x_d = nc.dram_tensor("xin", [D, SHALF + HIST], BF16, kind="ExternalInput").ap()
    w_d = nc.dram_tensor("wtab", [128, (K + 1) * NBLK], F32, kind="ExternalInput").ap()
    wd_d = nc.dram_tensor(
        "wdiag", [128, K * NBLK * 128], BF16, kind="ExternalInput"
    ).ap()
    o_d = nc.dram_tensor("out", [D, SHALF], BF16, kind="ExternalOutput").ap()

    with TileContext(nc) as tc:
        with (
            tc.tile_pool(name="const", bufs=1) as const,
            tc.tile_pool(name="xpool", bufs=6) as xpool,
            tc.psum_pool(name="ppool", bufs=8) as ppool,
            tc.tile_pool(name="opool", bufs=5) as opool,
        ):
            wsb = const.tile([128, (K + 1) * NBLK], F32, tag="wsb")
            nc.scalar.dma_start(out=wsb[:], in_=w_d)
            wdg = const.tile([128, K * NBLK * 128], BF16, tag="wdg")
            nc.scalar.dma_start(out=wdg[:], in_=wd_d)

            for _ in range(nreps):
                _emit_pass(nc, (xpool, ppool, opool), (x_d, o_d, wsb, wdg))

    nc.compile()
    _CACHE[nreps] = nc
    return nc


def _shard_inputs(x, weight, bias):
    import ml_dtypes

    bf16 = ml_dtypes.bfloat16
    x = np.asarray(x, dtype=np.float32)
    weight = np.asarray(weight, dtype=np.float32)
    bias = np.asarray(bias, dtype=np.float32)

    wr = weight[:, 0, :].reshape(NBLK, 128, K)          # [blk, p, k]
    # wtab[p, k*NBLK+blk] = w[blk*128+p, k]; wtab[p, K*NBLK+blk] = bias
    wtab = np.empty((128, (K + 1) * NBLK), dtype=np.float32)
    wtab[:, : K * NBLK] = wr.transpose(1, 2, 0).reshape(128, K * NBLK)
    wtab[:, K * NBLK :] = bias.reshape(NBLK, 128).T
    # wdiag[p, (k*NBLK+blk)*128 + j] = w[blk*128+p, k] * (j == p)
    wd = np.zeros((128, K * NBLK, 128), dtype=np.float32)
    pidx = np.arange(128)
    for k in range(K):
        for blk in range(NBLK):
            wd[pidx, k * NBLK + blk, pidx] = wr[blk, :, k]
    wdiag = wd.reshape(128, K * NBLK * 128).astype(bf16)

    in_maps = []
    for core in range(NCORES):
        b, h = divmod(core, 2)
        s0 = h * SHALF
        xc = np.empty((D, SHALF + HIST), dtype=bf16)
        xbt = x[b].T  # [D, S] view
        if s0 == 0:
            xc[:, :HIST] = 0.0
            xc[:, HIST:] = xbt[:, :SHALF]
        else:
            xc[:] = xbt[:, s0 - HIST : s0 + SHALF]
        in_maps.append({"xin": xc, "wtab": wtab, "wdiag": wdiag})
    return in_maps


def _run(x, weight, bias, trace=False):
    nc = _build_program()
    in_maps = _shard_inputs(x, weight, bias)
    res = run_bass_kernel_spmd(nc, in_maps, list(range(NCORES)), trace=trace)
    out = np.empty((B, S, D), dtype=np.float32)
    for core in range(NCORES):
        b, h = divmod(core, 2)
        out[b, h * SHALF : (h + 1) * SHALF, :] = (
            res.results[core]["out"].astype(np.float32).T
        )
    return out, res


def kernel(x, weight, bias):
    out, _ = _run(x, weight, bias, trace=False)
    return out
